# revision 32
# baseline (speedup 1.0000x reference)
"""RVQTokenizer Trainium2 kernel.

Pipeline per core (256 of 2048 batch items, pure data parallel on 8 cores):
  encode: conv1d(1->256,k=3) + relu  ->  conv1d(256->128,k=3) + relu -> mean
  rvq:    12 sequential stages of (distance, argmin, gather, residual update)

Precision strategy: conv matmuls run on the PE in float32r (tf32-grade input
rounding) using a 2-pass hi/lo weight split, which restores ~fp32 weight
precision at 2 cycles/row instead of fp32's 4.  The RVQ distance/gather
matmuls are plain fp32 (exact codeword gather via one-hot matmul).  Distances
are computed as (2*r@cb^T) - ||c||^2 at small magnitude, skipping the
row-constant ||r||^2 term, which keeps the argmin ordering at ~1e-9 noise.
"""

import os
import numpy as np

import concourse.bass as bass
import concourse.mybir as mybir
from concourse.bass_utils import run_bass_kernel_spmd
from concourse.tile import TileContext

F32 = mybir.dt.float32
F32R = mybir.dt.float32r
U32 = mybir.dt.uint32
I32 = mybir.dt.int32
AF = mybir.ActivationFunctionType
OP = mybir.AluOpType
AX = mybir.AxisListType

B, FEAT, HID, LAT, NQ, NEMB = 2048, 840, 256, 128, 12, 512
NCORES = 8
IPC = B // NCORES          # items per core = 256
W = FEAT                   # 840
WPAD = W + 2               # 842
H0, H1N = 512, W - 512     # conv1 psum bank split: 512 + 328
NCH = HID // 128           # 2 channel chunks
NCC = NEMB // 128          # 4 code chunks
QMODE = os.environ.get("RVQT_QMODE", "f32r2")   # f32r2 | f32 | f32r1
NITEMS = int(os.environ.get("RVQT_ITEMS", IPC))


def _tf32(a):
    """Round fp32 to the PE's float32r input precision (11 explicit mantissa
    bits, round-to-nearest) so on-chip values match host bytes exactly."""
    b = np.ascontiguousarray(np.asarray(a, np.float32)).view(np.uint32)
    b = (b + np.uint32(0x800)) & np.uint32(0xFFFFF000)
    return b.view(np.float32)


def _legalize_waits(nc, max_waits=1):
    """This walrus build rejects >1 sync-wait per instruction: hoist extra
    waits onto NoOps inserted just before, on the same engine."""
    n = 0
    for fn in nc.m.functions:
        for bb in fn.blocks:
            insts = bb.instructions
            new, changed = [], False
            for inst in insts:
                si = inst.sync_info
                waits = list(si.on_wait) if si and si.on_wait else []
                if len(waits) > max_waits:
                    changed = True
                    for w in waits[:-max_waits]:
                        n += 1
                        nop = mybir.InstNoOp(name=f"I-waitfix-{n}", ins=[], outs=[])
                        nop.engine = inst.engine
                        nop.sync_info = mybir.SyncInfo(on_wait=[w], on_update=[])
                        new.append(nop)
                    si.on_wait = waits[-max_waits:]
                new.append(inst)
            if changed:
                bb.instructions = new
    return n


_STRIPPABLE = (
    mybir.InstMatmult, mybir.InstActivation, mybir.InstTensorTensor,
    mybir.InstTensorCopy, mybir.InstTensorScalarPtr, mybir.InstMax,
    mybir.InstMaxIndex, mybir.InstMemset,
)


def _strip_dead_updates(nc):
    """Tile puts a +1 sem update on every compute instruction; each serialized
    inc costs ~26 ns on the issuing engine.  Updates whose cumulative count is
    never referenced by any wait are dead: remove them and renumber later
    waits on the same semaphore."""
    n_stripped = 0
    for fn in nc.m.functions:
        blocks = fn.blocks
        per_bb = [bb.instructions for bb in blocks]
        insts = [i for lst in per_bb for i in lst]
        upds, refs, unsafe = {}, {}, set()
        for inst in insts:
            si = inst.sync_info
            if not si:
                continue
            for w in (si.on_wait or []):
                if (w.sync_type == "semaphore" and w.wait_mode == "sem-ge-imm"
                        and w.wait_reg is None):
                    refs.setdefault(w.id, set()).add(w.wait_value)
                else:
                    unsafe.add(getattr(w, "id", None))
            for j, u in enumerate(si.on_update or []):
                if (u.sync_type == "semaphore" and u.update_mode == "sem-inc"
                        and u.update_value == 1 and u.update_reg is None):
                    upds.setdefault(u.id, []).append((inst, j))
                else:
                    unsafe.add(u.id)
        remap = {}
        to_remove = {}
        for sid, lst in upds.items():
            if sid in unsafe:
                continue
            r = refs.get(sid, set())
            keep = []
            for j, (inst, _) in enumerate(lst):
                keep.append((j + 1) in r or not isinstance(inst, _STRIPPABLE))
            keep[-1] = True
            newc, k = [], 0
            for j in range(len(lst)):
                if keep[j]:
                    k += 1
                newc.append(k)
            remap[sid] = newc
            for j, (inst, uidx) in enumerate(lst):
                if not keep[j]:
                    to_remove.setdefault(id(inst), (inst, []))[1].append(uidx)
                    n_stripped += 1
        for _, (inst, idxs) in to_remove.items():
            si = inst.sync_info
            ups = list(si.on_update)
            for uidx in sorted(idxs, reverse=True):
                del ups[uidx]
            si.on_update = ups
        for inst in insts:
            si = inst.sync_info
            if not si or not si.on_wait:
                continue
            ws = list(si.on_wait)
            changed = False
            for w in ws:
                if (w.sync_type == "semaphore" and w.wait_mode == "sem-ge-imm"
                        and w.wait_reg is None and w.id in remap):
                    v = w.wait_value
                    nc_map = remap[w.id]
                    if 1 <= v <= len(nc_map):
                        nv = nc_map[v - 1]
                        if nv != v:
                            w.wait_value = nv
                            changed = True
            if changed:
                si.on_wait = ws
        for bb, lst in zip(blocks, per_bb):
            bb.instructions = lst
    return n_stripped


def _window3(row_ap, w):
    """[[1,3],[1,w]] view: 3 partition-rows reading the same DRAM row at
    offsets 0/1/2 (the k=3 conv taps)."""
    ap = row_ap.unsqueeze(0).copy()
    lst = ap.ap
    lst[0] = [1, 3]
    ap.ap = lst
    return ap


def build_module():
    nc = bass.Bass("TRN2", target_bir_lowering=False, debug=False, num_devices=1)
    conv_dt = F32 if QMODE == "f32" else F32R
    npass = 2 if QMODE == "f32r2" else 1

    # ---- DRAM I/O (per core) ----
    xpad = nc.dram_tensor("xpad", [IPC, WPAD], conv_dt, kind="ExternalInput")
    # conv1: K = 3 taps x npass (hi/lo folded into the contraction dim)
    w1s = nc.dram_tensor("w1s", [3 * npass, HID], conv_dt, kind="ExternalInput")
    w2t = nc.dram_tensor("w2t", [128, npass * 3 * NCH * 128], conv_dt,
                         kind="ExternalInput")
    b1d = nc.dram_tensor("b1d", [128, NCH], F32, kind="ExternalInput")
    b2d = nc.dram_tensor("b2d", [128, 1], F32, kind="ExternalInput")
    cbt2 = nc.dram_tensor("cbt2", [LAT, NQ * NEMB], F32, kind="ExternalInput")  # 2*cb^T
    cbg = nc.dram_tensor("cbg", [128, NQ * NCC * LAT], F32, kind="ExternalInput")
    csqb = nc.dram_tensor("csqb", [NQ * NEMB], F32, kind="ExternalInput")       # ||c||^2
    niota = nc.dram_tensor("niota", [128, NCC * IPC], F32, kind="ExternalInput")
    ident = nc.dram_tensor("ident", [128, 128], F32, kind="ExternalInput")
    zq_o = nc.dram_tensor("zq", [IPC, LAT], F32, kind="ExternalOutput")
    idx_o = nc.dram_tensor("idx", [IPC, NQ], I32, kind="ExternalOutput")

    inv_w = float(np.float32(1.0) / np.float32(W))

    with TileContext(nc) as tc:
        with tc.tile_pool(name="const", bufs=1) as cpool, \
             tc.tile_pool(name="rtp", bufs=1) as rtpool:
            # ---- constants to SBUF ----
            w1t_s = cpool.tile([3 * npass, HID], conv_dt, tag="w1")
            nc.sync.dma_start(w1t_s[:], w1s[:])
            w2t_s = cpool.tile([128, npass * 3 * NCH * 128], conv_dt, tag="w2")
            nc.sync.dma_start(w2t_s[:], w2t[:])
            b1_s = cpool.tile([128, NCH], F32, tag="b1")
            nc.sync.dma_start(b1_s[:], b1d[:])
            b2_s = cpool.tile([128, 1], F32, tag="b2")
            nc.sync.dma_start(b2_s[:], b2d[:])
            cbt2_s = cpool.tile([128, NQ * NEMB], F32, tag="cbt2")
            cbg_s = cpool.tile([128, NQ * NCC * LAT], F32, tag="cbg")
            csqb_s = cpool.tile([128, NQ * NEMB], F32, tag="csqb")
            niota_s = cpool.tile([128, NCC * IPC], F32, tag="niota")
            ident_s = cpool.tile([128, 128], F32, tag="ident")
            nc.sync.dma_start(ident_s[:], ident[:])
            ones_s = cpool.tile([1, 128], F32R, tag="ones")
            nc.vector.memset(ones_s[:].bitcast(F32), 1.0)

            # residual r^T [lat, items], one tile per 128-item chunk so
            # chunk 0's RVQ can overlap chunk 1's encode
            rT = [rtpool.tile([128, 128], F32, tag=f"rT{c}", name=f"rT{c}")
                  for c in range(2)]
            zqT = [rtpool.tile([128, 128], F32, tag=f"zqT{c}", name=f"zqT{c}")
                   for c in range(2)]
            idxall = [rtpool.tile([128, NQ], U32, tag=f"idxall{c}", name=f"idxall{c}")
                      for c in range(2)]

            # ================= encode + rvq (interleaved) =================
            with tc.tile_pool(name="xs", bufs=4) as xpool, \
                 tc.tile_pool(name="h1", bufs=3) as h1pool, \
                 tc.tile_pool(name="h2r", bufs=2) as h2rpool, \
                 tc.tile_pool(name="zs", bufs=2) as zspool, \
                 tc.tile_pool(name="dneg", bufs=2) as dnpool, \
                 tc.tile_pool(name="mx", bufs=2) as mxpool, \
                 tc.tile_pool(name="oh", bufs=2) as ohpool, \
                 tc.tile_pool(name="irow", bufs=2) as irpool, \
                 tc.tile_pool(name="psc1", bufs=2, space="PSUM") as psc1, \
                 tc.tile_pool(name="psc2", bufs=3, space="PSUM") as psc2, \
                 tc.tile_pool(name="psrv", bufs=1, space="PSUM") as psrv:

                def conv1_block(i):
                    xs = xpool.tile([3 * npass, W], conv_dt, tag="xs")
                    for p in range(npass):
                        nc.sync.dma_start(xs[3 * p: 3 * p + 3, :],
                                          _window3(xpad[i, 0:W], W))
                    h1p = [h1pool.tile([128, WPAD], conv_dt, tag=f"h1p{c}", name=f"h1p{c}")
                           for c in range(NCH)]
                    for c in range(NCH):
                        ps1a = psc1.tile([128, H0], F32, tag="c1a")
                        ps1b = psc1.tile([128, W - H0], F32, tag="c1b", bufs=1)
                        nc.tensor.matmul(ps1a[:], w1t_s[:, c * 128: (c + 1) * 128],
                                         xs[:, 0:H0], start=True, stop=True)
                        nc.tensor.matmul(ps1b[:], w1t_s[:, c * 128: (c + 1) * 128],
                                         xs[:, H0:W], start=True, stop=True)
                        # relu(h+b1) eviction, split across ACT and DVE
                        nc.scalar.activation(
                            h1p[c][:, 1: 1 + H0], ps1a[:], AF.Relu,
                            bias=b1_s[:, c: c + 1], scale=1.0,
                        )
                        nc.vector.tensor_scalar(
                            h1p[c][:, 1 + H0: 1 + W], ps1b[:],
                            b1_s[:, c: c + 1], 0.0, op0=OP.add, op1=OP.max,
                        )
                        nc.gpsimd.memset(h1p[c][:, 0:1].bitcast(F32), 0.0)
                        nc.gpsimd.memset(h1p[c][:, W + 1: W + 2].bitcast(F32), 0.0)
                    return h1p

                def conv2_block(i, h1p):
                    h2r = h2rpool.tile([128, W], F32, tag="h2r")
                    zsum = zspool.tile([128, 2], F32, tag="zsum")
                    for hh, (lo, hi) in enumerate(((0, 420), (420, W))):
                        ps2 = psc2.tile([128, 420], F32, tag="h2")
                        first = True
                        for p in range(npass):
                            for c in range(NCH):
                                for k in range(3):
                                    woff = ((p * 3 + k) * NCH + c) * 128
                                    nc.tensor.matmul(
                                        ps2[:],
                                        w2t_s[:, woff: woff + 128],
                                        h1p[c][:, k + lo: k + hi],
                                        start=first,
                                        stop=(p == npass - 1 and c == NCH - 1 and k == 2),
                                    )
                                    first = False
                        # relu(h2+b2) + free-dim sum in one ACT op
                        nc.scalar.activation(
                            h2r[:, lo:hi], ps2[:], AF.Relu,
                            bias=b2_s[:], scale=1.0,
                            accum_out=zsum[:, hh: hh + 1],
                        )
                    # z[:, i] = (sum0 + sum1) / 840
                    nc.vector.tensor_scalar(
                        rT[i // 128][:, i % 128: i % 128 + 1],
                        zsum[:, 0:1], zsum[:, 1:2], inv_w,
                        op0=OP.add, op1=OP.mult,
                    )

                def rvq_const_dma(sq):
                    sl = slice(sq * NEMB, (sq + 1) * NEMB)
                    nc.sync.dma_start(cbt2_s[:, sl], cbt2[:, sl])
                    nc.sync.dma_start(
                        cbg_s[:, sq * NCC * LAT:(sq + 1) * NCC * LAT],
                        cbg[:, sq * NCC * LAT:(sq + 1) * NCC * LAT])
                    nc.sync.dma_start(
                        csqb_s[:, sl],
                        csqb[sl].unsqueeze(0).broadcast_to((128, NEMB)),
                    )

                def rvq_stage(ch, sq):
                    r = rT[ch]
                    dps = psrv.tile([128, NEMB], F32, tag="dta")
                    nc.tensor.matmul(
                        dps[:], r[:],
                        cbt2_s[:, sq * NEMB: (sq + 1) * NEMB],
                        start=True, stop=True,
                    )
                    dn = dnpool.tile([128, NEMB], F32, tag="dn")
                    nc.vector.tensor_tensor(
                        dn[:], dps[:], csqb_s[:, sq * NEMB: (sq + 1) * NEMB],
                        op=OP.subtract,
                    )
                    mx8 = mxpool.tile([128, 8], F32, tag="mx8")
                    ix8 = mxpool.tile([128, 8], U32, tag="ix8")
                    nc.vector.max(mx8[:], dn[:])
                    nc.vector.max_index(ix8[:], mx8[:], dn[:])
                    nc.vector.tensor_copy(idxall[ch][:, sq: sq + 1], ix8[:, 0:1])
                    idxf = mxpool.tile([128, 1], F32, tag="idxf")
                    nc.vector.tensor_copy(idxf[:], ix8[:, 0:1])
                    tps = psrv.tile([1, 128], F32, tag="dta")
                    nc.tensor.transpose(tps[:], idxf[:], ident_s[:])
                    idxrn = irpool.tile([1, 128], F32R, tag="idxrn")
                    nc.scalar.activation(idxrn[:], tps[:], AF.Copy,
                                         bias=0.0, scale=-1.0)
                    Dps = psrv.tile([128, 128], F32, tag="dta")
                    nc.tensor.matmul(Dps[:], ones_s[:], idxrn[:],
                                     start=True, stop=True)
                    qps = psrv.tile([128, 128], F32, tag="qps")
                    for cc in range(NCC):
                        oh = ohpool.tile([128, 128], F32, tag=f"oh{cc % 2}",
                                         name=f"oh{cc % 2}")
                        nc.vector.tensor_tensor(
                            oh[:], Dps[:], niota_s[:, cc * IPC: cc * IPC + 128],
                            op=OP.is_equal,
                        )
                        goff = (sq * NCC + cc) * LAT
                        nc.tensor.matmul(
                            qps[:], cbg_s[:, goff: goff + LAT], oh[:],
                            start=(cc == 0), stop=(cc == NCC - 1),
                        )
                    if sq < NQ - 1:
                        nc.vector.tensor_tensor(r[:], r[:], qps[:], op=OP.subtract)
                    if sq == 0:
                        nc.vector.tensor_copy(zqT[ch][:], qps[:])
                    else:
                        nc.vector.tensor_tensor(zqT[ch][:], zqT[ch][:], qps[:],
                                                op=OP.add)

                # ---- emission schedule ----
                # chunk-0 encode; then chunk-1 encode with chunk-0 RVQ stages
                # interleaved (RVQ is item-parallel); then chunk-1 RVQ.
                nc.sync.dma_start(niota_s[:], niota[:])
                half = min(128, NITEMS)
                h1prev = conv1_block(0)
                for i in range(half):
                    h1next = conv1_block(i + 1) if i + 1 < NITEMS else None
                    conv2_block(i, h1prev)
                    h1prev = h1next
                rest = list(range(half, NITEMS))
                stages = list(range(NQ))
                gap = max(1, len(rest) // (NQ + 1)) if rest else 1
                si = 0
                for j, i in enumerate(rest):
                    if j % gap == 0 and si < NQ:
                        rvq_const_dma(si)
                        rvq_stage(0, si)
                        si += 1
                    conv2_block(i, h1prev)
                    h1prev = conv1_block(i + 1) if i + 1 < NITEMS else None
                while si < NQ:
                    rvq_const_dma(si)
                    rvq_stage(0, si)
                    si += 1
                for sq in range(NQ):
                    rvq_stage(1, sq)

                # ---- outputs ----
                for ch in range(2):
                    tq = psrv.tile([128, 128], F32, tag="dta")
                    nc.tensor.transpose(tq[:], zqT[ch][:], ident_s[:])
                    zrow = dnpool.tile([128, 128], F32, tag="zrow")
                    nc.scalar.copy(zrow[:], tq[:])
                    nc.sync.dma_start(zq_o[ch * 128: (ch + 1) * 128, :], zrow[:])
                    nc.sync.dma_start(
                        idx_o[ch * 128: (ch + 1) * 128, :],
                        idxall[ch][:].bitcast(I32),
                    )

    if os.environ.get("RVQT_STRIP", "1") == "1":
        _strip_dead_updates(nc)
    _legalize_waits(nc)
    return nc


_CACHE = {}


def _prep_inputs(x, w1, b1, w2, b2, codebooks):
    """Host-side packing shared by all cores (weights) + per-core x shards."""
    x = np.asarray(x, np.float32)
    w1 = np.asarray(w1, np.float32)
    b1 = np.asarray(b1, np.float32)
    w2 = np.asarray(w2, np.float32)
    b2 = np.asarray(b2, np.float32)
    cb = np.asarray(codebooks, np.float32)

    npass = 2 if QMODE == "f32r2" else 1
    if QMODE == "f32":
        w1p = w1[None]
        w2p = w2[None]
        xr = x
    else:
        w1hi = _tf32(w1)
        w2hi = _tf32(w2)
        xr = _tf32(x)
        if npass == 2:
            w1p = np.stack([w1hi, _tf32(w1 - w1hi)])
            w2p = np.stack([w2hi, _tf32(w2 - w2hi)])
        else:
            w1p = w1hi[None]
            w2p = w2hi[None]

    # w1s[3p+k, c] = w1p[p, c, 0, k]  (hi/lo passes stacked in contraction)
    w1s = np.empty((3 * npass, HID), np.float32)
    for p in range(npass):
        for k in range(3):
            w1s[3 * p + k] = w1p[p, :, 0, k]
    # w2t[a, ((p*3+k)*NCH+c)*128+m] = w2p[p, m, c*128+a, k]
    w2t = np.ascontiguousarray(
        w2p.reshape(npass, 128, NCH, 128, 3)        # p, m, c, a, k
           .transpose(3, 0, 4, 2, 1)                # a, p, k, c, m
           .reshape(128, npass * 3 * NCH * 128)
    )
    b1r = np.ascontiguousarray(b1.reshape(NCH, 128).T)    # [128, NCH]
    b2r = np.ascontiguousarray(b2.reshape(128, 1))
    cb64 = cb.astype(np.float64)
    # cbt2[l, q*NEMB+e] = 2*cb[q,e,l]
    cbt2 = np.ascontiguousarray(
        (2.0 * cb).transpose(2, 0, 1).reshape(LAT, NQ * NEMB)
    )
    # cbg[e, (q*NCC+c)*LAT+l] = cb[q, c*128+e, l]
    cbg = np.ascontiguousarray(
        cb.reshape(NQ, NCC, 128, LAT).transpose(2, 0, 1, 3)
          .reshape(128, NQ * NCC * LAT)
    )
    csq = ((cb64 ** 2).sum(axis=2)).astype(np.float32).reshape(NQ * NEMB)
    niota = np.ascontiguousarray(np.broadcast_to(
        -np.arange(NEMB, dtype=np.float32).reshape(NCC, 128)
          .T.reshape(128, NCC, 1), (128, NCC, IPC)).reshape(128, NCC * IPC))
    ident = np.eye(128, dtype=np.float32)

    xp = np.zeros((B, WPAD), np.float32)
    xp[:, 1:-1] = xr
    shared = dict(w1s=w1s, w2t=w2t, b1d=b1r, b2d=b2r, cbt2=cbt2, cbg=cbg,
                  csqb=csq, niota=niota, ident=ident)
    in_maps = []
    for c in range(NCORES):
        m = dict(shared)
        m["xpad"] = np.ascontiguousarray(xp[c * IPC:(c + 1) * IPC])
        in_maps.append(m)
    return in_maps


def kernel(x, w1, b1, w2, b2, codebooks, _trace=False):
    if "nc" not in _CACHE:
        _CACHE["nc"] = build_module()
    nc = _CACHE["nc"]
    in_maps = _prep_inputs(x, w1, b1, w2, b2, codebooks)
    try:
        res = run_bass_kernel_spmd(nc, in_maps, list(range(NCORES)), trace=_trace)
    except ModuleNotFoundError:
        res = run_bass_kernel_spmd(nc, in_maps, list(range(NCORES)), trace=False)
    zq = np.concatenate([res.results[c]["zq"] for c in range(NCORES)], axis=0)
    idx = np.concatenate([res.results[c]["idx"] for c in range(NCORES)], axis=0)
    out = (zq[:, None, :].astype(np.float32), idx[:, None, :].astype(np.int32))
    if _trace:
        return out, res
    return out


# revision 33
# speedup vs baseline: 1.0721x; 1.0721x over previous
"""RVQTokenizer Trainium2 kernel.

Pipeline per core (256 of 2048 batch items, pure data parallel on 8 cores):
  encode: conv1d(1->256,k=3) + relu  ->  conv1d(256->128,k=3) + relu -> mean
  rvq:    12 sequential stages of (distance, argmin, gather, residual update)

Precision strategy: conv matmuls run on the PE in float32r (tf32-grade input
rounding) using a 2-pass hi/lo weight split, which restores ~fp32 weight
precision at 2 cycles/row instead of fp32's 4.  The RVQ distance/gather
matmuls are plain fp32 (exact codeword gather via one-hot matmul).  Distances
are computed as (2*r@cb^T) - ||c||^2 at small magnitude, skipping the
row-constant ||r||^2 term, which keeps the argmin ordering at ~1e-9 noise.
"""

import os
import numpy as np

import concourse.bass as bass
import concourse.mybir as mybir
from concourse.bass_utils import run_bass_kernel_spmd
from concourse.tile import TileContext

F32 = mybir.dt.float32
F32R = mybir.dt.float32r
U32 = mybir.dt.uint32
I32 = mybir.dt.int32
AF = mybir.ActivationFunctionType
OP = mybir.AluOpType
AX = mybir.AxisListType

B, FEAT, HID, LAT, NQ, NEMB = 2048, 840, 256, 128, 12, 512
NCORES = 8
IPC = B // NCORES          # items per core = 256
W = FEAT                   # 840
WPAD = W + 2               # 842
H0, H1N = 512, W - 512     # conv1 psum bank split: 512 + 328
NCH = HID // 128           # 2 channel chunks
NCC = NEMB // 128          # 4 code chunks
QMODE = os.environ.get("RVQT_QMODE", "f32r2")   # f32r2 | f32 | f32r1
NITEMS = int(os.environ.get("RVQT_ITEMS", IPC))


def _tf32(a):
    """Round fp32 to the PE's float32r input precision (11 explicit mantissa
    bits, round-to-nearest) so on-chip values match host bytes exactly."""
    b = np.ascontiguousarray(np.asarray(a, np.float32)).view(np.uint32)
    b = (b + np.uint32(0x800)) & np.uint32(0xFFFFF000)
    return b.view(np.float32)


def _legalize_waits(nc, max_waits=1):
    """This walrus build rejects >1 sync-wait per instruction: hoist extra
    waits onto NoOps inserted just before, on the same engine."""
    n = 0
    mm_max = int(os.environ.get("RVQT_MMWAITS", "1"))
    for fn in nc.m.functions:
        for bb in fn.blocks:
            insts = bb.instructions
            new, changed = [], False
            for inst in insts:
                max_waits = mm_max if isinstance(inst, mybir.InstMatmult) else 1
                si = inst.sync_info
                waits = list(si.on_wait) if si and si.on_wait else []
                if len(waits) > max_waits:
                    changed = True
                    for w in waits[:-max_waits]:
                        n += 1
                        nop = mybir.InstNoOp(name=f"I-waitfix-{n}", ins=[], outs=[])
                        nop.engine = inst.engine
                        nop.sync_info = mybir.SyncInfo(on_wait=[w], on_update=[])
                        new.append(nop)
                    si.on_wait = waits[-max_waits:]
                new.append(inst)
            if changed:
                bb.instructions = new
    return n


_STRIPPABLE = (
    mybir.InstMatmult, mybir.InstActivation, mybir.InstTensorTensor,
    mybir.InstTensorCopy, mybir.InstTensorScalarPtr, mybir.InstMax,
    mybir.InstMaxIndex, mybir.InstMemset,
)


def _strip_dead_updates(nc):
    """Tile puts a +1 sem update on every compute instruction; each serialized
    inc costs ~26 ns on the issuing engine.  Updates whose cumulative count is
    never referenced by any wait are dead: remove them and renumber later
    waits on the same semaphore."""
    n_stripped = 0
    for fn in nc.m.functions:
        blocks = fn.blocks
        per_bb = [bb.instructions for bb in blocks]
        insts = [i for lst in per_bb for i in lst]
        upds, refs, unsafe = {}, {}, set()
        for inst in insts:
            si = inst.sync_info
            if not si:
                continue
            for w in (si.on_wait or []):
                if (w.sync_type == "semaphore" and w.wait_mode == "sem-ge-imm"
                        and w.wait_reg is None):
                    refs.setdefault(w.id, set()).add(w.wait_value)
                else:
                    unsafe.add(getattr(w, "id", None))
            for j, u in enumerate(si.on_update or []):
                if (u.sync_type == "semaphore" and u.update_mode == "sem-inc"
                        and u.update_value == 1 and u.update_reg is None):
                    upds.setdefault(u.id, []).append((inst, j))
                else:
                    unsafe.add(u.id)
        remap = {}
        to_remove = {}
        for sid, lst in upds.items():
            if sid in unsafe:
                continue
            r = refs.get(sid, set())
            keep = []
            for j, (inst, _) in enumerate(lst):
                keep.append((j + 1) in r or not isinstance(inst, _STRIPPABLE))
            keep[-1] = True
            newc, k = [], 0
            for j in range(len(lst)):
                if keep[j]:
                    k += 1
                newc.append(k)
            remap[sid] = newc
            for j, (inst, uidx) in enumerate(lst):
                if not keep[j]:
                    to_remove.setdefault(id(inst), (inst, []))[1].append(uidx)
                    n_stripped += 1
        for _, (inst, idxs) in to_remove.items():
            si = inst.sync_info
            ups = list(si.on_update)
            for uidx in sorted(idxs, reverse=True):
                del ups[uidx]
            si.on_update = ups
        for inst in insts:
            si = inst.sync_info
            if not si or not si.on_wait:
                continue
            ws = list(si.on_wait)
            changed = False
            for w in ws:
                if (w.sync_type == "semaphore" and w.wait_mode == "sem-ge-imm"
                        and w.wait_reg is None and w.id in remap):
                    v = w.wait_value
                    nc_map = remap[w.id]
                    if 1 <= v <= len(nc_map):
                        nv = nc_map[v - 1]
                        if nv != v:
                            w.wait_value = nv
                            changed = True
            if changed:
                si.on_wait = ws
        for bb, lst in zip(blocks, per_bb):
            bb.instructions = lst
    return n_stripped


def _window3(row_ap, w):
    """[[1,3],[1,w]] view: 3 partition-rows reading the same DRAM row at
    offsets 0/1/2 (the k=3 conv taps)."""
    ap = row_ap.unsqueeze(0).copy()
    lst = ap.ap
    lst[0] = [1, 3]
    ap.ap = lst
    return ap


def build_module():
    nc = bass.Bass("TRN2", target_bir_lowering=False, debug=False, num_devices=1)
    conv_dt = F32 if QMODE == "f32" else F32R
    npass = 2 if QMODE == "f32r2" else 1

    # ---- DRAM I/O (per core) ----
    xpad = nc.dram_tensor("xpad", [IPC, WPAD], conv_dt, kind="ExternalInput")
    # conv1: K = 3 taps x npass (hi/lo folded into the contraction dim)
    w1s = nc.dram_tensor("w1s", [3 * npass, HID], conv_dt, kind="ExternalInput")
    w2t = nc.dram_tensor("w2t", [128, npass * 3 * NCH * 128], conv_dt,
                         kind="ExternalInput")
    b1d = nc.dram_tensor("b1d", [128, NCH], F32, kind="ExternalInput")
    b2d = nc.dram_tensor("b2d", [128, 1], F32, kind="ExternalInput")
    cbt2 = nc.dram_tensor("cbt2", [LAT, NQ * NEMB], F32, kind="ExternalInput")  # 2*cb^T
    cbg = nc.dram_tensor("cbg", [128, NQ * NCC * LAT], F32, kind="ExternalInput")
    csqb = nc.dram_tensor("csqb", [NQ * NEMB], F32, kind="ExternalInput")       # ||c||^2
    niota = nc.dram_tensor("niota", [128, NCC * IPC], F32, kind="ExternalInput")
    ident = nc.dram_tensor("ident", [128, 128], F32, kind="ExternalInput")
    zq_o = nc.dram_tensor("zq", [IPC, LAT], F32, kind="ExternalOutput")
    idx_o = nc.dram_tensor("idx", [IPC, NQ], I32, kind="ExternalOutput")

    inv_w = float(np.float32(1.0) / np.float32(W))

    with TileContext(nc) as tc:
        with tc.tile_pool(name="const", bufs=1) as cpool, \
             tc.tile_pool(name="rtp", bufs=1) as rtpool:
            # ---- constants to SBUF ----
            w1t_s = cpool.tile([3 * npass, HID], conv_dt, tag="w1")
            nc.sync.dma_start(w1t_s[:], w1s[:])
            w2t_s = cpool.tile([128, npass * 3 * NCH * 128], conv_dt, tag="w2")
            nc.sync.dma_start(w2t_s[:], w2t[:])
            b1_s = cpool.tile([128, NCH], F32, tag="b1")
            nc.sync.dma_start(b1_s[:], b1d[:])
            b2_s = cpool.tile([128, 1], F32, tag="b2")
            nc.sync.dma_start(b2_s[:], b2d[:])
            cbt2_s = cpool.tile([128, NQ * NEMB], F32, tag="cbt2")
            cbg_s = cpool.tile([128, NQ * NCC * LAT], F32, tag="cbg")
            csqb_s = cpool.tile([128, NQ * NEMB], F32, tag="csqb")
            niota_s = cpool.tile([128, NCC * IPC], F32, tag="niota")
            ident_s = cpool.tile([128, 128], F32, tag="ident")
            nc.sync.dma_start(ident_s[:], ident[:])
            ones_s = cpool.tile([1, 128], F32R, tag="ones")
            nc.vector.memset(ones_s[:].bitcast(F32), 1.0)

            # residual r^T [lat, items]; built column-by-column by encode
            rT = rtpool.tile([128, IPC], F32, tag="rT")
            zqT = rtpool.tile([128, IPC], F32, tag="zqT")
            idxall = [rtpool.tile([128, NQ], U32, tag=f"idxall{c}", name=f"idxall{c}")
                      for c in range(2)]

            # ================= encode =================
            with tc.tile_pool(name="xs", bufs=4) as xpool, \
                 tc.tile_pool(name="h1", bufs=3) as h1pool, \
                 tc.tile_pool(name="h2r", bufs=2) as h2rpool, \
                 tc.tile_pool(name="zs", bufs=2) as zspool, \
                 tc.tile_pool(name="psc1", bufs=2, space="PSUM") as psc1, \
                 tc.tile_pool(name="psc2", bufs=4, space="PSUM") as psc2:
                def conv1_block(i):
                    xs = xpool.tile([3 * npass, W], conv_dt, tag="xs")
                    for p in range(npass):
                        nc.sync.dma_start(xs[3 * p: 3 * p + 3, :],
                                          _window3(xpad[i, 0:W], W))
                    h1p = [h1pool.tile([128, WPAD], conv_dt, tag=f"h1p{c}", name=f"h1p{c}")
                           for c in range(NCH)]
                    for c in range(NCH):
                        ps1 = psc1.tile([128, W], F32, tag="c1")
                        for sp, (lo, hi) in enumerate(((0, H0), (H0, W))):
                            nc.tensor.matmul(
                                ps1[:, lo:hi],
                                w1t_s[:, c * 128: (c + 1) * 128],
                                xs[:, lo:hi],
                                start=True, stop=True,
                            )
                        # relu(h+b1) eviction, split across ACT and DVE
                        nc.scalar.activation(
                            h1p[c][:, 1: 1 + H0], ps1[:, 0:H0], AF.Relu,
                            bias=b1_s[:, c: c + 1], scale=1.0,
                        )
                        nc.vector.tensor_scalar(
                            h1p[c][:, 1 + H0: 1 + W], ps1[:, H0:W],
                            b1_s[:, c: c + 1], 0.0, op0=OP.add, op1=OP.max,
                        )
                        nc.gpsimd.memset(h1p[c][:, 0:1].bitcast(F32), 0.0)
                        nc.gpsimd.memset(h1p[c][:, W + 1: W + 2].bitcast(F32), 0.0)
                    return h1p

                def conv2_block(i, h1p):
                    h2r = h2rpool.tile([128, W], F32, tag="h2r")
                    zsum = zspool.tile([128, 2], F32, tag="zsum")
                    for hh, (lo, hi) in enumerate(((0, 420), (420, W))):
                        ps2 = psc2.tile([128, 420], F32, tag="h2")
                        first = True
                        for p in range(npass):
                            for c in range(NCH):
                                for k in range(3):
                                    woff = ((p * 3 + k) * NCH + c) * 128
                                    nc.tensor.matmul(
                                        ps2[:],
                                        w2t_s[:, woff: woff + 128],
                                        h1p[c][:, k + lo: k + hi],
                                        start=first,
                                        stop=(p == npass - 1 and c == NCH - 1 and k == 2),
                                    )
                                    first = False
                        # relu(h2+b2) + free-dim sum in one ACT op
                        nc.scalar.activation(
                            h2r[:, lo:hi], ps2[:], AF.Relu,
                            bias=b2_s[:], scale=1.0,
                            accum_out=zsum[:, hh: hh + 1],
                        )
                    # z[:, i] = (sum0 + sum1) / 840
                    nc.vector.tensor_scalar(
                        rT[:, i: i + 1], zsum[:, 0:1], zsum[:, 1:2], inv_w,
                        op0=OP.add, op1=OP.mult,
                    )

                # software pipeline: conv1 for item i+1 is emitted before
                # conv2 for item i, so h1p evictions overlap PE work
                h1prev = conv1_block(0)
                for i in range(NITEMS):
                    h1next = conv1_block(i + 1) if i + 1 < NITEMS else None
                    conv2_block(i, h1prev)
                    h1prev = h1next

            # ================= rvq =================
            with tc.tile_pool(name="dneg", bufs=2) as dnpool, \
                 tc.tile_pool(name="mx", bufs=2) as mxpool, \
                 tc.tile_pool(name="oh", bufs=2) as ohpool, \
                 tc.tile_pool(name="irow", bufs=2) as irpool, \
                 tc.tile_pool(name="psd", bufs=2, space="PSUM") as psd, \
                 tc.tile_pool(name="pst", bufs=1, space="PSUM") as pst, \
                 tc.tile_pool(name="psD", bufs=1, space="PSUM") as psD, \
                 tc.tile_pool(name="psq", bufs=2, space="PSUM") as psq:
                # RVQ constants stream in per-stage slices here (after the
                # encode emission) so they don't head-of-line block the
                # encode input DMAs at kernel start.
                nc.sync.dma_start(niota_s[:], niota[:])
                for s in range(NQ):
                    sl = slice(s * NEMB, (s + 1) * NEMB)
                    nc.sync.dma_start(cbt2_s[:, sl], cbt2[:, sl])
                    nc.sync.dma_start(cbg_s[:, s * NCC * LAT:(s + 1) * NCC * LAT],
                                      cbg[:, s * NCC * LAT:(s + 1) * NCC * LAT])
                    nc.sync.dma_start(
                        csqb_s[:, sl],
                        csqb[sl].unsqueeze(0).broadcast_to((128, NEMB)),
                    )
                for s in range(NQ):
                    idxrn = irpool.tile([1, IPC], F32R, tag="idxrn")
                    for ch in range(2):
                        dps = psd.tile([128, NEMB], F32, tag="dps")
                        nc.tensor.matmul(
                            dps[:], rT[:, ch * 128: (ch + 1) * 128],
                            cbt2_s[:, s * NEMB: (s + 1) * NEMB],
                            start=True, stop=True,
                        )
                        dn = dnpool.tile([128, NEMB], F32, tag="dn")
                        nc.vector.tensor_tensor(
                            dn[:], dps[:], csqb_s[:, s * NEMB: (s + 1) * NEMB],
                            op=OP.subtract,
                        )
                        mx8 = mxpool.tile([128, 8], F32, tag="mx8")
                        ix8 = mxpool.tile([128, 8], U32, tag="ix8")
                        nc.vector.max(mx8[:], dn[:])
                        nc.vector.max_index(ix8[:], mx8[:], dn[:])
                        nc.vector.tensor_copy(idxall[ch][:, s: s + 1], ix8[:, 0:1])
                        idxf = mxpool.tile([128, 1], F32, tag="idxf")
                        nc.vector.tensor_copy(idxf[:], ix8[:, 0:1])
                        tps = pst.tile([1, 128], F32, tag="tps")
                        nc.tensor.transpose(tps[:], idxf[:], ident_s[:])
                        nc.scalar.activation(
                            idxrn[:, ch * 128: (ch + 1) * 128], tps[:],
                            AF.Copy, bias=0.0, scale=-1.0,
                        )
                    Dps = psD.tile([128, IPC], F32, tag="Dps")
                    nc.tensor.matmul(Dps[:], ones_s[:], idxrn[:], start=True, stop=True)
                    qps = psq.tile([128, IPC], F32, tag="qps")
                    for cc in range(NCC):
                        oh = ohpool.tile([128, IPC], F32, tag=f"oh{cc % 2}")
                        nc.vector.tensor_tensor(
                            oh[:], Dps[:], niota_s[:, cc * IPC: (cc + 1) * IPC],
                            op=OP.is_equal,
                        )
                        goff = (s * NCC + cc) * LAT
                        nc.tensor.matmul(
                            qps[:], cbg_s[:, goff: goff + LAT], oh[:],
                            start=(cc == 0), stop=(cc == NCC - 1),
                        )
                    if s < NQ - 1:
                        nc.vector.tensor_tensor(rT[:], rT[:], qps[:], op=OP.subtract)
                    if s == 0:
                        nc.vector.tensor_copy(zqT[:], qps[:])
                    else:
                        nc.vector.tensor_tensor(zqT[:], zqT[:], qps[:], op=OP.add)

                # ---- outputs ----
                for ch in range(2):
                    tq = psD.tile([128, 128], F32, tag="tq")
                    nc.tensor.transpose(tq[:], zqT[:, ch * 128: (ch + 1) * 128],
                                        ident_s[:])
                    zrow = dnpool.tile([128, 128], F32, tag="zrow")
                    nc.scalar.copy(zrow[:], tq[:])
                    nc.sync.dma_start(zq_o[ch * 128: (ch + 1) * 128, :], zrow[:])
                    nc.sync.dma_start(
                        idx_o[ch * 128: (ch + 1) * 128, :],
                        idxall[ch][:].bitcast(I32),
                    )

    if os.environ.get("RVQT_STRIP", "1") == "1":
        _strip_dead_updates(nc)
    _legalize_waits(nc)
    return nc


_CACHE = {}


def _prep_inputs(x, w1, b1, w2, b2, codebooks):
    """Host-side packing shared by all cores (weights) + per-core x shards."""
    x = np.asarray(x, np.float32)
    w1 = np.asarray(w1, np.float32)
    b1 = np.asarray(b1, np.float32)
    w2 = np.asarray(w2, np.float32)
    b2 = np.asarray(b2, np.float32)
    cb = np.asarray(codebooks, np.float32)

    npass = 2 if QMODE == "f32r2" else 1
    if QMODE == "f32":
        w1p = w1[None]
        w2p = w2[None]
        xr = x
    else:
        w1hi = _tf32(w1)
        w2hi = _tf32(w2)
        xr = _tf32(x)
        if npass == 2:
            w1p = np.stack([w1hi, _tf32(w1 - w1hi)])
            w2p = np.stack([w2hi, _tf32(w2 - w2hi)])
        else:
            w1p = w1hi[None]
            w2p = w2hi[None]

    # w1s[3p+k, c] = w1p[p, c, 0, k]  (hi/lo passes stacked in contraction)
    w1s = np.empty((3 * npass, HID), np.float32)
    for p in range(npass):
        for k in range(3):
            w1s[3 * p + k] = w1p[p, :, 0, k]
    # w2t[a, ((p*3+k)*NCH+c)*128+m] = w2p[p, m, c*128+a, k]
    w2t = np.ascontiguousarray(
        w2p.reshape(npass, 128, NCH, 128, 3)        # p, m, c, a, k
           .transpose(3, 0, 4, 2, 1)                # a, p, k, c, m
           .reshape(128, npass * 3 * NCH * 128)
    )
    b1r = np.ascontiguousarray(b1.reshape(NCH, 128).T)    # [128, NCH]
    b2r = np.ascontiguousarray(b2.reshape(128, 1))
    cb64 = cb.astype(np.float64)
    # cbt2[l, q*NEMB+e] = 2*cb[q,e,l]
    cbt2 = np.ascontiguousarray(
        (2.0 * cb).transpose(2, 0, 1).reshape(LAT, NQ * NEMB)
    )
    # cbg[e, (q*NCC+c)*LAT+l] = cb[q, c*128+e, l]
    cbg = np.ascontiguousarray(
        cb.reshape(NQ, NCC, 128, LAT).transpose(2, 0, 1, 3)
          .reshape(128, NQ * NCC * LAT)
    )
    csq = ((cb64 ** 2).sum(axis=2)).astype(np.float32).reshape(NQ * NEMB)
    niota = np.ascontiguousarray(np.broadcast_to(
        -np.arange(NEMB, dtype=np.float32).reshape(NCC, 128)
          .T.reshape(128, NCC, 1), (128, NCC, IPC)).reshape(128, NCC * IPC))
    ident = np.eye(128, dtype=np.float32)

    xp = np.zeros((B, WPAD), np.float32)
    xp[:, 1:-1] = xr
    shared = dict(w1s=w1s, w2t=w2t, b1d=b1r, b2d=b2r, cbt2=cbt2, cbg=cbg,
                  csqb=csq, niota=niota, ident=ident)
    in_maps = []
    for c in range(NCORES):
        m = dict(shared)
        m["xpad"] = np.ascontiguousarray(xp[c * IPC:(c + 1) * IPC])
        in_maps.append(m)
    return in_maps


def kernel(x, w1, b1, w2, b2, codebooks, _trace=False):
    if "nc" not in _CACHE:
        _CACHE["nc"] = build_module()
    nc = _CACHE["nc"]
    in_maps = _prep_inputs(x, w1, b1, w2, b2, codebooks)
    try:
        res = run_bass_kernel_spmd(nc, in_maps, list(range(NCORES)), trace=_trace)
    except ModuleNotFoundError:
        res = run_bass_kernel_spmd(nc, in_maps, list(range(NCORES)), trace=False)
    zq = np.concatenate([res.results[c]["zq"] for c in range(NCORES)], axis=0)
    idx = np.concatenate([res.results[c]["idx"] for c in range(NCORES)], axis=0)
    out = (zq[:, None, :].astype(np.float32), idx[:, None, :].astype(np.int32))
    if _trace:
        return out, res
    return out
